# revision 11
# baseline (speedup 1.0000x reference)
"""Causal single-head attention on 8 Trainium2 NeuronCores.

Math: out[b] = softmax(causal((x_b Wq^T)(x_b Wk^T)^T / 8)) @ (x_b Wv^T)

Strategy (pure batch data-parallelism, 512 batches/core):
  - Host precomputes A = (Wq^T Wk)/8 AND g = x @ A, shipping both x^T and
    g^T pair-packed bf16 in one contiguous DRAM tensor (2KB/partition
    lines -> full-rate DMA, no on-device projection or PSUM round trip
    for g).
  - Per 8-batch group on device:
      v    = x^T-stationary @ blockdiag(WvT,WvT)      (4 matmuls, [t,h])
      sT   = x_b^T @ g_b^T per batch -> scores^T in 2 PSUM banks
      mask : bank A += -48 * tril_strict via matmul(lhsT=LM, rhs=I) on PE;
             bank B is masked post-exp by a 0/1 triangle multiply on DVE
             (splits mask cost across two otherwise-idle slots)
      expT = ACT exp(scores - 3) over all 8 batches in one instruction
      U|Z  = expT-stationary @ [v | ones]             (8 matmuls)
      U|Z staged to fp16 SBUF by the Pool engine, DMAed out every 2
      groups (Z,U scale by e^-3 which cancels in the host-side U/Z).
  - DMAs are 2-group batched and issued from the SP queue.
"""

import sys

sys.path.insert(0, "/opt/trn_rl_repo")

import numpy as np

B, T, C, H = 4096, 128, 64, 64
NCORES = 8
BPC = B // NCORES          # 512 batches per core
PAIRS = BPC // 2           # 256
GROUPS = PAIRS // 4        # 64 groups of 4 pairs (8 batches)
NEG = -48.0                # causal mask additive constant (bank A)
EBIAS = -3.0               # exp bias: keeps U,Z in fp16 range; cancels in U/Z

_cache = {}

import os
DBG_OUT_F32 = os.environ.get("K_OUT_F32", "0") == "1"
DBG_SINGLE_DMA = os.environ.get("K_SINGLE_DMA", "0") == "1"
DBG_NO_BIAS = os.environ.get("K_NO_BIAS", "0") == "1"
DBG_SOB_DVE = os.environ.get("K_SOB_DVE", "0") == "1"


def _build(dtype_bf16):
    import concourse.bass as bass
    import concourse.bacc as bacc
    import concourse.mybir as mybir
    import concourse.tile as tile

    f32 = mybir.dt.float32
    bf16 = mybir.dt.bfloat16
    fp16 = mybir.dt.float16

    nc = bacc.Bacc("TRN2", target_bir_lowering=False, debug=False,
                   num_devices=NCORES)

    # xg[g] = [x^T pair-packed (512 cols) | g^T pair-packed (512 cols)]
    xg = nc.dram_tensor("xg", [GROUPS, 128, 1024], bf16, kind="ExternalInput")
    # consts packed in one tensor: wvt2 | lm | ident | tri
    cpk = nc.dram_tensor("cpk", [128, 1280], bf16, kind="ExternalInput")
    out_dt = f32 if DBG_OUT_F32 else fp16
    uzout = nc.dram_tensor("uzout", [GROUPS // 2, 128, 1040], out_dt,
                           kind="ExternalOutput")

    def scol(b):
        return 512 * (b % 2) + 128 * (b // 2)

    with tile.TileContext(nc) as tc:
        with (
            tc.tile_pool(name="const", bufs=1) as cpool,
            tc.tile_pool(name="sbx", bufs=2) as sbx,
            tc.tile_pool(name="sbe", bufs=2) as sbe,
            tc.tile_pool(name="sbo", bufs=2) as sbo,
            tc.tile_pool(name="psv", bufs=2, space=bass.MemorySpace.PSUM) as psv,
            tc.tile_pool(name="pss", bufs=2, space=bass.MemorySpace.PSUM) as pss,
        ):
            cts = cpool.tile([128, 1280], bf16, tag="cts")
            nc.sync.dma_start(cts[:], cpk[:])
            c_wvt = cts[:, 0:128]
            c_lm = cts[:, 128:256]
            c_id = cts[:, 256:768]
            c_tri = cts[:, 768:1280]

            ebias = cpool.tile([128, 1], f32, tag="ebias")
            nc.vector.memset(ebias[:], EBIAS)

            # persistent v|ones tiles (double-buffered by hand); the ones
            # columns are written once and never touched again
            vo_a = cpool.tile([128, 520], bf16, tag="voa")
            vo_b = cpool.tile([128, 520], bf16, tag="vob")
            vo_bufs = [vo_a, vo_b]
            for vb in vo_bufs:
                nc.vector.memset(vb[:], 1.0)

            for j in range(GROUPS // 2):
                sxg = sbx.tile([128, 2048], bf16, tag="sxg")
                if DBG_SINGLE_DMA:
                    nc.sync.dma_start(sxg[:, 0:1024], xg[2 * j])
                    nc.sync.dma_start(sxg[:, 1024:2048], xg[2 * j + 1])
                else:
                    nc.sync.dma_start(
                        sxg[:].rearrange("p (g c) -> p g c", g=2),
                        xg[2 * j:2 * j + 2].rearrange("g p c -> p g c"))

                so2 = sbo.tile([128, 1040], out_dt, tag="so2")

                for h in range(2):
                    g = 2 * j + h
                    xs = sxg[:, 1024 * h:1024 * h + 512]
                    gs = sxg[:, 1024 * h + 512:1024 * h + 1024]

                    # v via blockdiag(WvT,WvT): [t, h] pair-packed
                    pgv = psv.tile([128, 512], f32, tag="pgv")
                    for p in range(4):
                        nc.tensor.matmul(
                            pgv[:, 128 * p:128 * (p + 1)],
                            xs[:, 128 * p:128 * (p + 1)], c_wvt,
                            start=True, stop=True)

                    # scores^T[s, t]; batch b%2 picks the PSUM bank so
                    # concurrent sub-array matmuls never share a bank
                    ps = pss.tile([128, 1024], f32, tag="ps")
                    for b in range(8):
                        p, hf = b // 2, b % 2
                        xTb = xs[64 * hf:64 * (hf + 1), 128 * p:128 * (p + 1)]
                        gTb = gs[64 * hf:64 * (hf + 1), 128 * p:128 * (p + 1)]
                        nc.tensor.matmul(
                            ps[:, scol(b):scol(b) + 128], xTb, gTb,
                            start=(b < 2), stop=False,
                            skip_group_check=True)
                    # causal mask accumulate: += -48 * 1[s > t], one per bank
                    for bank in range(2):
                        nc.tensor.matmul(
                            ps[:, 512 * bank:512 * (bank + 1)], c_lm, c_id,
                            start=False, stop=True,
                            skip_group_check=True)

                    se = sbe.tile([128, 1024], bf16, tag="se")
                    if DBG_NO_BIAS:
                        nc.scalar.activation(se[:], ps[:],
                                             mybir.ActivationFunctionType.Exp)
                    else:
                        nc.scalar.activation(se[:], ps[:],
                                             mybir.ActivationFunctionType.Exp,
                                             bias=ebias[:, 0:1])

                    vo = vo_bufs[h]
                    vo3 = vo[:].rearrange("p (b c) -> p b c", c=65)
                    nc.vector.tensor_copy(
                        vo3[:, :, 0:64],
                        pgv[:].rearrange("p (b c) -> p b c", c=64))

                    # U|Z back into ps (scores are consumed): [t, 65] per batch
                    for b in range(8):
                        lhsT = se[:, scol(b):scol(b) + 128]
                        col = 65 * b if b < 7 else 512
                        nc.tensor.matmul(
                            ps[:, col:col + 65], lhsT,
                            vo[:, 65 * b:65 * (b + 1)],
                            start=True, stop=True,
                            skip_group_check=True)

                    # compact U|Z -> fp16 staging, split across DVE and ACT
                    nc.vector.tensor_copy(so2[:, 520 * h:520 * h + 455],
                                          ps[:, 0:455])
                    if DBG_SOB_DVE:
                        nc.vector.tensor_copy(
                            so2[:, 520 * h + 455:520 * h + 520],
                            ps[:, 512:577])
                    else:
                        nc.scalar.copy(so2[:, 520 * h + 455:520 * h + 520],
                                       ps[:, 512:577])

                nc.sync.dma_start(uzout[j], so2[:])

    nc.compile()
    return nc


def _make_in_maps(x, Wq, Wk, Wv):
    import ml_dtypes

    bf = ml_dtypes.bfloat16
    x = np.asarray(x, dtype=np.float32)
    A = (np.asarray(Wq, np.float32).T @ np.asarray(Wk, np.float32)) / np.sqrt(H)

    wvT = np.asarray(Wv, np.float32).T
    wvt2 = np.zeros((128, 128), np.float32)
    wvt2[0:64, 0:64] = wvT
    wvt2[64:128, 64:128] = wvT

    k_idx = np.arange(128)[:, None]
    s_idx = np.arange(128)[None, :]
    lm = np.where(s_idx > k_idx, np.float32(NEG), np.float32(0.0))
    ident = np.tile(np.eye(128, dtype=np.float32), (1, 4))
    tri01 = np.tile((k_idx <= s_idx).astype(np.float32), (1, 4))

    cpk = np.concatenate([wvt2, lm, ident, tri01], axis=1).astype(bf)

    # [B, T, C] -> xT [B, C, T]; pair-pack 2 batches on the partition dim
    xt = np.ascontiguousarray(x.transpose(0, 2, 1)).astype(bf)
    xt = xt.reshape(NCORES, GROUPS, 4, 128, 128)
    g = x @ A                                     # [B, T, C'] in f32
    gt = np.ascontiguousarray(g.transpose(0, 2, 1)).astype(bf)
    gt = gt.reshape(NCORES, GROUPS, 4, 128, 128)

    # per group: [128 part, 512 x-cols | 512 g-cols], contiguous per line
    xg_all = np.empty((NCORES, GROUPS, 128, 1024), bf)
    xg_all[:, :, :, 0:512] = xt.transpose(0, 1, 3, 2, 4).reshape(
        NCORES, GROUPS, 128, 512)
    xg_all[:, :, :, 512:1024] = gt.transpose(0, 1, 3, 2, 4).reshape(
        NCORES, GROUPS, 128, 512)

    return [dict(cpk=cpk, xg=np.ascontiguousarray(xg_all[i]))
            for i in range(NCORES)]


def kernel(x, Wq, Wk, Wv):
    from concourse.bass_utils import run_bass_kernel_spmd

    if "nc" not in _cache:
        _cache["nc"] = _build(True)
    nc = _cache["nc"]

    in_maps = _make_in_maps(x, Wq, Wk, Wv)
    res = run_bass_kernel_spmd(nc, in_maps, list(range(NCORES)))

    out = np.empty((B, T, H), np.float32)
    for i in range(NCORES):
        uzr = np.asarray(res.results[i]["uzout"], np.float32)  # [32,128,1040]
        uz = uzr.reshape(GROUPS // 2, 128, 2, 520)
        uz = np.moveaxis(uz, 2, 1).reshape(GROUPS, 128, 8, 65)
        uz = np.moveaxis(uz, 2, 1).reshape(BPC, 128, 65)
        out[i * BPC:(i + 1) * BPC] = uz[:, :, 0:64] / uz[:, :, 64:65]
    return out
